# revision 33
# baseline (speedup 1.0000x reference)
"""Cross-attention layer on 8 Trainium2 NeuronCores (Bass/Tile SPMD).

Sharding: tensor-parallel over heads. Each core projects Q^T/K^T/V for its
4 heads (fp8e4 DoubleRow matmuls, fp32 accumulate, weights pre-scaled x64 on
host so they clear the fp8 subnormal range; descale folded into the PSUM
evacuation), runs masked softmax attention in bf16 transposed layout, then
four AllToAlls (one per 512-token half-batch, each issued as soon as its
ctx^T columns are ready) redistribute ctx^T (fp8, x8 scaled) from
head-sharded to token-sharded; every core then runs the fp8 output
projection + residual + LayerNorm for its 64-token slice of each half.
The splits pipeline every collective under attention/O-proj compute.

Phase B is software-pipelined three deep (scores+exp+chunk-sum | row-sum
matmul+approx-reciprocal | broadcast+ctx+normalize) so the PE never waits
on the softmax normalization chain.
"""
import sys

sys.path.insert(0, "/opt/trn_rl_repo")

import numpy as np
import ml_dtypes

import concourse.bacc as bacc
import concourse.mybir as mybir
import concourse.tile as tile
from concourse.bass_utils import run_bass_kernel_spmd

BF16 = ml_dtypes.bfloat16
FP8 = ml_dtypes.float8_e4m3   # matches mybir.dt.float8e4 (max 240)

NCORES = 8
P = 128            # partitions / head dim
H = 4096
KT = H // P        # 32 k-tiles along any H contraction
NT = KT // 2       # 16 DoubleRow k-pair steps
NH = 32
NHL = NH // NCORES  # 4 local heads
CW = NHL * P       # 512 local c-columns
B = 2
LB = 1024          # tokens per batch
L2 = B * LB        # 2048 total tokens
TLH = 64           # tokens per core per (batch, half)
QW = 512           # token-block width
NQ = L2 // QW      # 4
NCH = 4            # a2a chunks: (batch, half)
MSK = -1e30
WS = 64.0          # fp8 weight pre-scale
CS = 8.0           # fp8 ctx pre-scale
SQ = 1.0 / np.sqrt(P)

_CACHE = {}

F32 = mybir.dt.float32
BF = mybir.dt.bfloat16
F8 = mybir.dt.float8e4
DR = mybir.MatmulPerfMode.DoubleRow


def _build(debug=False):
    nc = bacc.Bacc("TRN2", target_bir_lowering=False, debug=False,
                   num_devices=NCORES)

    hid_d = nc.dram_tensor("hid8", [P, NQ, KT, QW], F8, kind="ExternalInput")
    vis_d = nc.dram_tensor("vis8", [P, NQ, KT, QW], F8, kind="ExternalInput")
    wq_d = nc.dram_tensor("wq8", [P, KT, CW], F8, kind="ExternalInput")
    wk_d = nc.dram_tensor("wk8", [P, KT, CW], F8, kind="ExternalInput")
    wv_d = nc.dram_tensor("wv8", [P, KT, CW], F8, kind="ExternalInput")
    wo_d = nc.dram_tensor("wo8", [P, H // QW, KT, QW], F8, kind="ExternalInput")
    bqT_d = nc.dram_tensor("bqT", [P, NHL], F32, kind="ExternalInput")
    nmask_d = nc.dram_tensor("nmask", [1, B], F32, kind="ExternalInput")
    hb_d = nc.dram_tensor("hb", [B * 2 * TLH, H], F32, kind="ExternalInput")
    g_d = nc.dram_tensor("g", [1, H], F32, kind="ExternalInput")
    bta_d = nc.dram_tensor("bta", [1, H], F32, kind="ExternalInput")
    out_d = nc.dram_tensor("out", [B * 2 * TLH, H], F32, kind="ExternalOutput")
    if debug:
        qT_dbg = nc.dram_tensor("qT_dbg", [P, NHL * L2], BF, kind="ExternalOutput")
        kT_dbg = nc.dram_tensor("kT_dbg", [P, NHL * L2], BF, kind="ExternalOutput")
        v_dbg = nc.dram_tensor("v_dbg", [P, 16 * CW], BF, kind="ExternalOutput")
        ctxT_dbg = nc.dram_tensor("ctxT_dbg", [P, NHL * L2], F32,
                                  kind="ExternalOutput")
        octxT_dbg = nc.dram_tensor("octxT_dbg", [P, B * KT * 2 * TLH], F32,
                                   kind="ExternalOutput")
        xpre_dbg = nc.dram_tensor("xpre_dbg", [B * 2 * TLH, H], F32,
                                  kind="ExternalOutput")

    with tile.TileContext(nc) as tc:
        with tc.tile_pool(name="persist", bufs=1) as pers, \
             tc.tile_pool(name="dram", bufs=1, space="DRAM") as dram:

            pqkv = tc.alloc_tile_pool(name="pqkv", bufs=1)
            qT_sb = pqkv.tile([P, NHL, L2], BF)     # Q^T/sqrt(hd): [hd, h, l]
            kT_sb = pqkv.tile([P, NHL, L2], BF)     # K^T: [hd, h, v]
            v_sb = pqkv.tile([P, 16, CW], BF)       # V: [v, vt, c]
            ctxT_sb = pqkv.tile([P, NHL, L2], F8)   # ctx^T * CS: [hd, h, l]
            bqT_sb = pers.tile([P, NHL], F32)
            nmask_sb = pers.tile([1, B], F32)
            ones_bf = pers.tile([P, 1], BF)
            ones_f32 = pers.tile([1, P], F32)
            nc.sync.dma_start(out=bqT_sb[:], in_=bqT_d[:])
            nc.sync.dma_start(out=nmask_sb[:], in_=nmask_d[:])
            nc.vector.memset(ones_bf[:], 1.0)
            nc.vector.memset(ones_f32[:], 1.0)

            # ---------------- Phase A: Q^T, K^T, V projections (fp8 DR) ----
            pa = tc.alloc_tile_pool(name="phaseA", bufs=1)
            with tc.tile_pool(name="psA", bufs=6, space="PSUM") as psA:
                wq_sb = pa.tile([P, KT, CW], F8, name="wq_sb")
                wk_sb = pa.tile([P, KT, CW], F8, name="wk_sb")
                wv_sb = pa.tile([P, KT, CW], F8, name="wv_sb")
                for u in range(4):
                    nc.sync.dma_start(
                        out=wq_sb[:, u * (KT // 4):(u + 1) * (KT // 4), :],
                        in_=wq_d[:, u * (KT // 4):(u + 1) * (KT // 4), :])

                def qk_pair(xts, w_sb, b_sb, dst_sb, q0, scale):
                    for h in range(NHL):
                        pss = [psA.tile([P, QW], F32, tag="psA", name=f"ps{j}")
                               for j in range(2)]
                        for t in range(NT):
                            for j in range(2):
                                nc.tensor.matmul(
                                    pss[j][:],
                                    w_sb[:, 2 * t:2 * t + 2, h * P:(h + 1) * P],
                                    xts[j][:, 2 * t:2 * t + 2, :],
                                    start=(t == 0), stop=(t == NT - 1),
                                    perf_mode=DR)
                        for j in range(2):
                            dst = dst_sb[:, h, (q0 + j) * QW:(q0 + j + 1) * QW]
                            if b_sb is None:
                                nc.vector.tensor_scalar_mul(dst, pss[j][:], scale)
                            else:
                                nc.vector.tensor_scalar(
                                    out=dst, in0=pss[j][:],
                                    scalar1=scale, scalar2=b_sb[:, h:h + 1],
                                    op0=mybir.AluOpType.mult,
                                    op1=mybir.AluOpType.add)

                for qq in range(NQ // 2):
                    xts = []
                    for j in range(2):
                        xT = pa.tile([P, KT, QW], F8, tag="xT", bufs=4)
                        for u in range(4):
                            nc.sync.dma_start(
                                out=xT[:, u * (KT // 4):(u + 1) * (KT // 4), :],
                                in_=hid_d[:, 2 * qq + j,
                                          u * (KT // 4):(u + 1) * (KT // 4), :])
                        xts.append(xT)
                    if qq == 0:
                        nc.sync.dma_start(out=wk_sb[:], in_=wk_d[:])
                        nc.sync.dma_start(out=wv_sb[:], in_=wv_d[:])
                    qk_pair(xts, wq_sb, bqT_sb, qT_sb, 2 * qq, SQ / WS)

                # K and V share the vis block loads
                for qq in range(NQ // 2):
                    xts = []
                    for j in range(2):
                        xT = pa.tile([P, KT, QW], F8, tag="xT", bufs=4)
                        nc.sync.dma_start(out=xT[:], in_=vis_d[:, 2 * qq + j])
                        xts.append(xT)
                    qk_pair(xts, wk_sb, None, kT_sb, 2 * qq, 1.0 / WS)
                    for j in range(2):
                        for vt in range(4):
                            g_vt = (2 * qq + j) * 4 + vt
                            ps = psA.tile([P, CW], F32, tag="psA")
                            for t in range(NT):
                                nc.tensor.matmul(
                                    ps[:],
                                    xts[j][:, 2 * t:2 * t + 2, vt * P:(vt + 1) * P],
                                    wv_sb[:, 2 * t:2 * t + 2, :],
                                    start=(t == 0), stop=(t == NT - 1),
                                    perf_mode=DR)
                            nc.vector.tensor_scalar_mul(
                                v_sb[:, g_vt, :], ps[:], 1.0 / WS)
            pa.release()

            # ------------- Phase B + C pipelined per (batch, half) ---------
            pb = tc.alloc_tile_pool(name="phaseB", bufs=1)
            pc = tc.alloc_tile_pool(name="phaseC", bufs=1)
            ps = tc.alloc_tile_pool(name="psBC", bufs=1, space="PSUM")

            a2a_in = [dram.tile([NCORES, P, NHL, TLH], F8, name=f"a2a_in{i}")
                      for i in range(NCH)]
            a2a_out = [dram.tile([NCORES, P, NHL, TLH], F8, name=f"a2a_out{i}")
                       for i in range(NCH)]
            octxT = [None, None]

            # phase B software pipeline: stage1 = scores+exp+chunk-sum,
            # stage2 = row-sum matmul + reciprocal, stage3 = bcast+ctx+norm
            def b_stage1(b, h, lh):
                # exps are unmasked (masked vision rows are zeroed host-side,
                # so masked columns drop out of ctx); the 0/1 mask weights the
                # row-sum accumulation instead.
                qoff = b * LB + lh * QW
                attnT = pb.tile([P, 8, QW], BF, tag="attnT", bufs=3)
                for u in range(4):
                    sc_ps = ps.tile([P, 2, QW], F32, tag="sc", bufs=2)
                    for j in range(2):
                        vb = 2 * u + j
                        nc.tensor.matmul(
                            sc_ps[:, j, :],
                            kT_sb[:, h, b * LB + vb * P: b * LB + (vb + 1) * P],
                            qT_sb[:, h, qoff: qoff + QW],
                            start=True, stop=True)
                    nc.scalar.activation(
                        attnT[:, 2 * u:2 * u + 2, :], sc_ps[:],
                        mybir.ActivationFunctionType.Exp)
                S = pb.tile([P, QW], BF, tag="S", bufs=2)
                nc.vector.tensor_tensor(
                    out=S[:], in0=attnT[:, 0, :], in1=attnT[:, 1, :],
                    op=mybir.AluOpType.add)
                for vb in range(2, 8):
                    nc.vector.tensor_tensor(
                        out=S[:], in0=S[:], in1=attnT[:, vb, :],
                        op=mybir.AluOpType.add)
                return [b, h, lh, attnT, S, None]

            def b_stage2(st):
                b, S = st[0], st[4]
                rs_ps = ps.tile([1, QW], F32, tag="rs", bufs=1)
                nc.tensor.matmul(rs_ps[:], ones_bf[:], S[:],
                                 start=True, stop=True)
                rs_adj = pb.tile([1, QW], F32, tag="rsadj", bufs=2)
                nc.vector.tensor_scalar(
                    out=rs_adj[:], in0=rs_ps[:],
                    scalar1=nmask_sb[0:1, b:b + 1], scalar2=None,
                    op0=mybir.AluOpType.subtract)
                rcp_sb = pb.tile([1, QW], F32, tag="rcp", bufs=2)
                nc.vector.reciprocal_approx_fast(out=rcp_sb[:], in_=rs_adj[:])
                st[5] = rcp_sb

            def b_stage3(st):
                b, h, lh, attnT, S, rcp_sb = st
                qoff = b * LB + lh * QW
                rcp_rep = pb.tile([P, QW], F32, tag="rcprep", bufs=2)
                nc.gpsimd.partition_broadcast(rcp_rep[:], rcp_sb[:])
                ctx_ps = ps.tile([P, QW], F32, tag="ctx", bufs=1)
                for vb in range(8):
                    nc.tensor.matmul(
                        ctx_ps[:],
                        v_sb[:, b * 8 + vb, h * P:(h + 1) * P],
                        attnT[:, vb, :],
                        start=(vb == 0), stop=(vb == 7))
                nc.vector.scalar_tensor_tensor(
                    out=ctxT_sb[:, h, qoff: qoff + QW],
                    in0=ctx_ps[:], scalar=CS, in1=rcp_rep[:],
                    op0=mybir.AluOpType.mult, op1=mybir.AluOpType.mult)

            pipe = []

            def b_push(st):
                pipe.append(st)
                if len(pipe) >= 2 and pipe[-2][5] is None:
                    b_stage2(pipe[-2])
                if len(pipe) >= 3:
                    b_stage3(pipe.pop(0))

            def b_drain():
                while pipe:
                    st = pipe.pop(0)
                    if st[5] is None:
                        b_stage2(st)
                    b_stage3(st)

            def a2a_send(b, lh):
                ch = b * 2 + lh
                for h in range(NHL):
                    nc.sync.dma_start(
                        out=a2a_in[ch][:, :, h, :].rearrange("j p l -> p j l"),
                        in_=ctxT_sb[:, h, b * LB + lh * QW: b * LB + (lh + 1) * QW]
                            .rearrange("p (j l) -> p j l", j=NCORES))
                nc.gpsimd.collective_compute(
                    "AllToAll", mybir.AluOpType.bypass,
                    replica_groups=[list(range(NCORES))],
                    ins=[a2a_in[ch][:]], outs=[a2a_out[ch][:]])

            def a2a_recv(b, lh):
                ch = b * 2 + lh
                if octxT[b] is None:
                    octxT[b] = pc.tile([P, KT, 2 * TLH], F8, name=f"octxT{b}")
                for i in range(NCORES):
                    nc.sync.dma_start(
                        out=octxT[b][:, i * NHL:(i + 1) * NHL,
                                     lh * TLH:(lh + 1) * TLH],
                        in_=a2a_out[ch][i])

            def phaseB(b):
                for lh in range(2):
                    for h in range(NHL):
                        b_push(b_stage1(b, h, lh))
                    b_drain()
                    a2a_send(b, lh)

            def phaseC(b, g_sb, bta_sb):
                # O-projection for this batch's 128 tokens, all H columns
                hb_h = []
                for u in range(2):
                    hb_sb = pc.tile([P, H // 2], F32, tag=f"hb{u}", bufs=1)
                    nc.sync.dma_start(
                        out=hb_sb[:],
                        in_=hb_d[b * P:(b + 1) * P,
                                 u * (H // 2):(u + 1) * (H // 2)])
                    hb_h.append(hb_sb)
                x = pc.tile([P, H], F32, tag="x", bufs=2)
                msum = pc.tile([P, 8], F32, tag="msum", bufs=2)
                qsum = pc.tile([P, 8], F32, tag="qsum", bufs=2)
                for mc in range(H // QW):
                    wo_h = []
                    for u in range(2):
                        wo_sb = pc.tile([P, KT // 2, QW], F8, tag="wo", bufs=3)
                        nc.sync.dma_start(
                            out=wo_sb[:],
                            in_=wo_d[:, mc, u * (KT // 2):(u + 1) * (KT // 2), :])
                        wo_h.append(wo_sb)
                    po = ps.tile([P, QW], F32, tag="po", bufs=2)
                    for t in range(NT):
                        u, tu = divmod(t, NT // 2)
                        nc.tensor.matmul(
                            po[:],
                            octxT[b][:, 2 * t:2 * t + 2, :],
                            wo_h[u][:, 2 * tu:2 * tu + 2, :],
                            start=(t == 0), stop=(t == NT - 1),
                            perf_mode=DR)
                    hbu = hb_h[mc // 4]
                    hcs = slice((mc % 4) * QW, (mc % 4 + 1) * QW)
                    nc.vector.scalar_tensor_tensor(
                        out=x[:, mc * QW:(mc + 1) * QW],
                        in0=po[:], scalar=1.0 / (WS * CS),
                        in1=hbu[:, hcs],
                        op0=mybir.AluOpType.mult, op1=mybir.AluOpType.add,
                        accum_out=msum[:, mc:mc + 1])
                    nc.scalar.activation(
                        hbu[:, hcs], x[:, mc * QW:(mc + 1) * QW],
                        mybir.ActivationFunctionType.Square,
                        accum_out=qsum[:, mc:mc + 1])
                if debug:
                    nc.sync.dma_start(
                        out=xpre_dbg[b * P:(b + 1) * P, :], in_=x[:])
                # ---- LayerNorm over H for the 128 tokens ----
                musum = pc.tile([P, 1], F32, tag="musum", bufs=2)
                nc.scalar.activation(
                    msum[:], msum[:], mybir.ActivationFunctionType.Copy,
                    accum_out=musum[:])
                mu = pc.tile([P, 1], F32, tag="mu", bufs=2)
                nc.scalar.mul(mu[:], musum[:], 1.0 / H)
                ssq = pc.tile([P, 1], F32, tag="ssq", bufs=2)
                nc.scalar.activation(
                    qsum[:], qsum[:], mybir.ActivationFunctionType.Copy,
                    accum_out=ssq[:])
                mu2 = pc.tile([P, 1], F32, tag="mu2", bufs=2)
                nc.scalar.activation(mu2[:], mu[:],
                                     mybir.ActivationFunctionType.Square)
                var = pc.tile([P, 1], F32, tag="var", bufs=2)
                nc.vector.scalar_tensor_tensor(
                    out=var[:], in0=ssq[:], scalar=1.0 / H, in1=mu2[:],
                    op0=mybir.AluOpType.mult, op1=mybir.AluOpType.subtract)
                eps_sb = pc.tile([P, 1], F32, tag="eps", bufs=1)
                nc.vector.memset(eps_sb[:], 1e-5)
                std = pc.tile([P, 1], F32, tag="std", bufs=2)
                nc.scalar.activation(std[:], var[:],
                                     mybir.ActivationFunctionType.Sqrt,
                                     bias=eps_sb[:], scale=1.0)
                rstd = pc.tile([P, 1], F32, tag="rstd", bufs=2)
                nc.vector.reciprocal(rstd[:], std[:])
                # normalize + gain + bias in column halves, overlapping the
                # output DMA with the second half's DVE passes
                HH = H // 2
                for c in range(2):
                    cs = slice(c * HH, (c + 1) * HH)
                    nc.vector.tensor_scalar(
                        out=x[:, cs], in0=x[:, cs], scalar1=mu[:],
                        scalar2=rstd[:],
                        op0=mybir.AluOpType.subtract, op1=mybir.AluOpType.mult)
                    nc.vector.tensor_tensor(
                        out=x[:, cs], in0=x[:, cs], in1=g_sb[:, cs],
                        op=mybir.AluOpType.mult)
                    nc.vector.tensor_tensor(
                        out=x[:, cs], in0=x[:, cs], in1=bta_sb[:, cs],
                        op=mybir.AluOpType.add)
                    nc.sync.dma_start(
                        out=out_d[b * P:(b + 1) * P, c * HH:(c + 1) * HH],
                        in_=x[:, cs])

            # g/bta: load one row each (staged through the hb-tag buffer),
            # broadcast on the (idle) GpSimd engine
            g_sb = pc.tile([P, H], F32, name="g_sb")
            bta_sb = pc.tile([P, H], F32, name="bta_sb")
            for u in range(2):
                stage = pc.tile([P, H // 2], F32, tag=f"hb{u}", bufs=1)
                src = g_d if u == 0 else bta_d
                dst = g_sb if u == 0 else bta_sb
                nc.sync.dma_start(out=stage[0:1, :H // 2], in_=src[0:1, :H // 2])
                nc.gpsimd.partition_broadcast(dst[:, :H // 2],
                                              stage[0:1, :H // 2])
                nc.sync.dma_start(out=stage[0:1, :H // 2],
                                  in_=src[0:1, H // 2:])
                nc.gpsimd.partition_broadcast(dst[:, H // 2:],
                                              stage[0:1, :H // 2])

            phaseB(0)
            a2a_recv(0, 0)
            a2a_recv(0, 1)
            phaseB(1)
            if debug:
                nc.sync.dma_start(
                    out=qT_dbg[:], in_=qT_sb[:].rearrange("p h l -> p (h l)"))
                nc.sync.dma_start(
                    out=kT_dbg[:], in_=kT_sb[:].rearrange("p h l -> p (h l)"))
                nc.sync.dma_start(
                    out=v_dbg[:], in_=v_sb[:].rearrange("p t c -> p (t c)"))
                for h in range(NHL):
                    ctmp = pers.tile([P, L2], F32, tag="ctmp", bufs=2)
                    nc.scalar.copy(out=ctmp[:], in_=ctxT_sb[:, h, :])
                    nc.sync.dma_start(
                        out=ctxT_dbg[:, h * L2:(h + 1) * L2], in_=ctmp[:])
            phaseC(0, g_sb, bta_sb)
            a2a_recv(1, 0)
            a2a_recv(1, 1)
            if debug:
                for b in range(B):
                    for kq in range(4):
                        otmp = pers.tile([P, 8 * 2 * TLH], F32, tag="otmp",
                                         bufs=2)
                        nc.scalar.copy(
                            out=otmp[:],
                            in_=octxT[b][:, kq * 8:(kq + 1) * 8, :]
                                .rearrange("p k l -> p (k l)"))
                        nc.sync.dma_start(
                            out=octxT_dbg[:, (b * KT + kq * 8) * 2 * TLH:
                                          (b * KT + (kq + 1) * 8) * 2 * TLH],
                            in_=otmp[:])
            phaseC(1, g_sb, bta_sb)
            ps.release()
            pc.release()
            pb.release()
            pqkv.release()

    nc.compile()
    return nc


def _prep_inputs(hidden_states, vision_features, attention_mask,
                 Wq, bq, Wk, bk, Wv, bv, Wo, bo, ln_g, ln_b):
    f = np.asarray
    hs = f(hidden_states, dtype=np.float32).reshape(L2, H)
    vf = f(vision_features, dtype=np.float32).reshape(L2, H)
    am = f(attention_mask)
    Wq, bq = f(Wq, dtype=np.float32), f(bq, dtype=np.float32)
    Wk, bk = f(Wk, dtype=np.float32), f(bk, dtype=np.float32)
    Wv, bv = f(Wv, dtype=np.float32), f(bv, dtype=np.float32)
    Wo, bo = f(Wo, dtype=np.float32), f(bo, dtype=np.float32)
    ln_g, ln_b = f(ln_g, dtype=np.float32), f(ln_b, dtype=np.float32)

    def act_layout(x):  # [L2, H] -> [P, NQ, KT, QW] fp8
        return np.ascontiguousarray(
            x.T.reshape(KT, P, NQ, QW).transpose(1, 2, 0, 3)).astype(FP8)

    def w_layout(w_slice):  # [CW, H] -> [P, KT, CW] fp8, pre-scaled
        return np.ascontiguousarray(
            (w_slice.T * WS).reshape(KT, P, CW).transpose(1, 0, 2)).astype(FP8)

    hid8 = act_layout(hs)
    vfm = vf.copy().reshape(B, LB, H)
    vfm[np.asarray(am) == 0] = 0.0          # masked vision tokens drop out
    vis8 = act_layout(vfm.reshape(L2, H))
    wo8 = np.ascontiguousarray(
        (Wo.T * WS).reshape(KT, P, H // QW, QW).transpose(1, 2, 0, 3)).astype(FP8)
    nmask = (np.asarray(am) == 0).sum(axis=1).astype(np.float32).reshape(1, B)
    bo_eff = bo + Wo @ bv

    in_maps = []
    for c in range(NCORES):
        sl = slice(c * CW, (c + 1) * CW)
        # core c owns tokens b*LB + lh*QW + c*TLH + [0,TLH) per (b, lh)
        hb = np.empty((B * 2 * TLH, H), np.float32)
        for b in range(B):
            for lh in range(2):
                rows = hs[b * LB + lh * QW + c * TLH:
                          b * LB + lh * QW + (c + 1) * TLH]
                hb[(b * 2 + lh) * TLH:(b * 2 + lh + 1) * TLH] = rows + bo_eff
        in_maps.append({
            "hid8": hid8,
            "vis8": vis8,
            "wq8": w_layout(Wq[sl]),
            "wk8": w_layout(Wk[sl]),
            "wv8": w_layout(Wv[sl]),
            "wo8": wo8,
            "bqT": np.ascontiguousarray((bq[sl] * SQ).reshape(NHL, P).T),
            "nmask": nmask,
            "hb": hb,
            "g": np.ascontiguousarray(ln_g.reshape(1, H)),
            "bta": np.ascontiguousarray(ln_b.reshape(1, H)),
        })
    return in_maps


def kernel(**inputs) -> np.ndarray:
    key = "dbg" if inputs.pop("_debug", False) else "main"
    if key not in _CACHE:
        _CACHE[key] = _build(debug=(key == "dbg"))
    nc = _CACHE[key]
    in_maps = _prep_inputs(**inputs)
    res = run_bass_kernel_spmd(nc, in_maps, list(range(NCORES)))
    out = np.empty((B, LB, H), np.float32)
    for c in range(NCORES):
        o = res.results[c]["out"]
        for b in range(B):
            for lh in range(2):
                out[b, lh * QW + c * TLH: lh * QW + (c + 1) * TLH] = \
                    o[(b * 2 + lh) * TLH:(b * 2 + lh + 1) * TLH]
    if key == "dbg":
        kernel._dbg = res.results
    return out


# revision 34
# speedup vs baseline: 1.0436x; 1.0436x over previous
"""Cross-attention layer on 8 Trainium2 NeuronCores (Bass/Tile SPMD).

Sharding: tensor-parallel over heads. Each core projects Q^T/K^T/V for its
4 heads (fp8e4 DoubleRow matmuls, fp32 accumulate, weights pre-scaled x64 on
host so they clear the fp8 subnormal range; descale folded into the PSUM
evacuation), runs masked softmax attention in bf16 transposed layout, then
four AllToAlls (one per 512-token half-batch, each issued as soon as its
ctx^T columns are ready) redistribute ctx^T (fp8, x8 scaled) from
head-sharded to token-sharded; every core then runs the fp8 output
projection + residual + LayerNorm for its 64-token slice of each half.
The splits pipeline every collective under attention/O-proj compute.

Phase B is software-pipelined three deep (scores+exp+chunk-sum | row-sum
matmul+approx-reciprocal | broadcast+ctx+normalize) so the PE never waits
on the softmax normalization chain.
"""
import sys

sys.path.insert(0, "/opt/trn_rl_repo")

import numpy as np
import ml_dtypes

import concourse.bacc as bacc
import concourse.mybir as mybir
import concourse.tile as tile
from concourse.bass_utils import run_bass_kernel_spmd

BF16 = ml_dtypes.bfloat16
FP8 = ml_dtypes.float8_e4m3   # matches mybir.dt.float8e4 (max 240)

NCORES = 8
P = 128            # partitions / head dim
H = 4096
KT = H // P        # 32 k-tiles along any H contraction
NT = KT // 2       # 16 DoubleRow k-pair steps
NH = 32
NHL = NH // NCORES  # 4 local heads
CW = NHL * P       # 512 local c-columns
B = 2
LB = 1024          # tokens per batch
L2 = B * LB        # 2048 total tokens
TLH = 64           # tokens per core per (batch, half)
QW = 512           # token-block width
NQ = L2 // QW      # 4
NCH = 4            # a2a chunks: (batch, half)
MSK = -1e30
WS = 64.0          # fp8 weight pre-scale
CS = 8.0           # fp8 ctx pre-scale
SQ = 1.0 / np.sqrt(P)

_CACHE = {}

F32 = mybir.dt.float32
BF = mybir.dt.bfloat16
F8 = mybir.dt.float8e4
DR = mybir.MatmulPerfMode.DoubleRow


def _build(debug=False):
    nc = bacc.Bacc("TRN2", target_bir_lowering=False, debug=False,
                   num_devices=NCORES)

    hid_d = nc.dram_tensor("hid8", [P, NQ, KT, QW], F8, kind="ExternalInput")
    vis_d = nc.dram_tensor("vis8", [P, NQ, KT, QW], F8, kind="ExternalInput")
    wq_d = nc.dram_tensor("wq8", [P, KT, CW], F8, kind="ExternalInput")
    wk_d = nc.dram_tensor("wk8", [P, KT, CW], F8, kind="ExternalInput")
    wv_d = nc.dram_tensor("wv8", [P, KT, CW], F8, kind="ExternalInput")
    wo_d = nc.dram_tensor("wo8", [P, H // QW, KT, QW], F8, kind="ExternalInput")
    bqT_d = nc.dram_tensor("bqT", [P, NHL], F32, kind="ExternalInput")
    nmask_d = nc.dram_tensor("nmask", [1, B], F32, kind="ExternalInput")
    hb_d = nc.dram_tensor("hb", [B * 2 * TLH, H], F32, kind="ExternalInput")
    g_d = nc.dram_tensor("g", [1, H], F32, kind="ExternalInput")
    bta_d = nc.dram_tensor("bta", [1, H], F32, kind="ExternalInput")
    out_d = nc.dram_tensor("out", [B * 2 * TLH, H], F32, kind="ExternalOutput")
    if debug:
        qT_dbg = nc.dram_tensor("qT_dbg", [P, NHL * L2], BF, kind="ExternalOutput")
        kT_dbg = nc.dram_tensor("kT_dbg", [P, NHL * L2], BF, kind="ExternalOutput")
        v_dbg = nc.dram_tensor("v_dbg", [P, 16 * CW], BF, kind="ExternalOutput")
        ctxT_dbg = nc.dram_tensor("ctxT_dbg", [P, NHL * L2], F32,
                                  kind="ExternalOutput")
        octxT_dbg = nc.dram_tensor("octxT_dbg", [P, B * KT * 2 * TLH], F32,
                                   kind="ExternalOutput")
        xpre_dbg = nc.dram_tensor("xpre_dbg", [B * 2 * TLH, H], F32,
                                  kind="ExternalOutput")

    with tile.TileContext(nc) as tc:
        with tc.tile_pool(name="persist", bufs=1) as pers, \
             tc.tile_pool(name="dram", bufs=1, space="DRAM") as dram:

            pqkv = tc.alloc_tile_pool(name="pqkv", bufs=1)
            qT_sb = pqkv.tile([P, NHL, L2], BF)     # Q^T/sqrt(hd): [hd, h, l]
            kT_sb = pqkv.tile([P, NHL, L2], BF)     # K^T: [hd, h, v]
            v_sb = pqkv.tile([P, 16, CW], BF)       # V: [v, vt, c]
            ctxT_sb = pqkv.tile([P, NHL, L2], F8)   # ctx^T * CS: [hd, h, l]
            bqT_sb = pers.tile([P, NHL], F32)
            nmask_sb = pers.tile([1, B], F32)
            ones_bf = pers.tile([P, 1], BF)
            ones_f32 = pers.tile([1, P], F32)
            nc.sync.dma_start(out=bqT_sb[:], in_=bqT_d[:])
            nc.sync.dma_start(out=nmask_sb[:], in_=nmask_d[:])
            nc.vector.memset(ones_bf[:], 1.0)
            nc.vector.memset(ones_f32[:], 1.0)

            # ---------------- Phase A: Q^T, K^T, V projections (fp8 DR) ----
            pa = tc.alloc_tile_pool(name="phaseA", bufs=1)
            with tc.tile_pool(name="psA", bufs=6, space="PSUM") as psA:
                wq_sb = pa.tile([P, KT, CW], F8, name="wq_sb")
                wk_sb = pa.tile([P, KT, CW], F8, name="wk_sb")
                wv_sb = pa.tile([P, KT, CW], F8, name="wv_sb")
                for u in range(4):
                    nc.sync.dma_start(
                        out=wq_sb[:, u * (KT // 4):(u + 1) * (KT // 4), :],
                        in_=wq_d[:, u * (KT // 4):(u + 1) * (KT // 4), :])

                def qk_pair(xts, w_sb, b_sb, dst_sb, q0, scale):
                    for h in range(NHL):
                        pss = [psA.tile([P, QW], F32, tag="psA", name=f"ps{j}")
                               for j in range(2)]
                        for t in range(NT):
                            for j in range(2):
                                nc.tensor.matmul(
                                    pss[j][:],
                                    w_sb[:, 2 * t:2 * t + 2, h * P:(h + 1) * P],
                                    xts[j][:, 2 * t:2 * t + 2, :],
                                    start=(t == 0), stop=(t == NT - 1),
                                    perf_mode=DR)
                        for j in range(2):
                            dst = dst_sb[:, h, (q0 + j) * QW:(q0 + j + 1) * QW]
                            if b_sb is None:
                                nc.vector.tensor_scalar_mul(dst, pss[j][:], scale)
                            else:
                                nc.vector.tensor_scalar(
                                    out=dst, in0=pss[j][:],
                                    scalar1=scale, scalar2=b_sb[:, h:h + 1],
                                    op0=mybir.AluOpType.mult,
                                    op1=mybir.AluOpType.add)

                for qq in range(NQ // 2):
                    xts = []
                    for j in range(2):
                        xT = pa.tile([P, KT, QW], F8, tag="xT", bufs=4)
                        for u in range(4):
                            nc.sync.dma_start(
                                out=xT[:, u * (KT // 4):(u + 1) * (KT // 4), :],
                                in_=hid_d[:, 2 * qq + j,
                                          u * (KT // 4):(u + 1) * (KT // 4), :])
                        xts.append(xT)
                    if qq == 0:
                        nc.sync.dma_start(out=wk_sb[:], in_=wk_d[:])
                        nc.sync.dma_start(out=wv_sb[:], in_=wv_d[:])
                    qk_pair(xts, wq_sb, bqT_sb, qT_sb, 2 * qq, SQ / WS)

                # K and V share the vis block loads
                for qq in range(NQ // 2):
                    xts = []
                    for j in range(2):
                        xT = pa.tile([P, KT, QW], F8, tag="xT", bufs=4)
                        nc.sync.dma_start(out=xT[:], in_=vis_d[:, 2 * qq + j])
                        xts.append(xT)
                    qk_pair(xts, wk_sb, None, kT_sb, 2 * qq, 1.0 / WS)
                    for j in range(2):
                        for vt in range(4):
                            g_vt = (2 * qq + j) * 4 + vt
                            ps = psA.tile([P, CW], F32, tag="psA")
                            for t in range(NT):
                                nc.tensor.matmul(
                                    ps[:],
                                    xts[j][:, 2 * t:2 * t + 2, vt * P:(vt + 1) * P],
                                    wv_sb[:, 2 * t:2 * t + 2, :],
                                    start=(t == 0), stop=(t == NT - 1),
                                    perf_mode=DR)
                            nc.vector.tensor_scalar_mul(
                                v_sb[:, g_vt, :], ps[:], 1.0 / WS)
            pa.release()

            # ------------- Phase B + C pipelined per (batch, half) ---------
            pb = tc.alloc_tile_pool(name="phaseB", bufs=1)
            pc = tc.alloc_tile_pool(name="phaseC", bufs=1)
            ps = tc.alloc_tile_pool(name="psB", bufs=1, space="PSUM")
            psC = [None]

            a2a_in = [dram.tile([NCORES, P, NHL, TLH], F8, name=f"a2a_in{i}")
                      for i in range(NCH)]
            a2a_out = [dram.tile([NCORES, P, NHL, TLH], F8, name=f"a2a_out{i}")
                       for i in range(NCH)]
            octxT = [None, None]

            # phase B software pipeline: stage1 = scores+exp+chunk-sum,
            # stage2 = row-sum matmul + reciprocal, stage3 = bcast+ctx+norm
            def b_stage1(b, h, lh):
                # exps are unmasked (masked vision rows are zeroed host-side,
                # so masked columns drop out of ctx); the 0/1 mask weights the
                # row-sum accumulation instead.
                qoff = b * LB + lh * QW
                attnT = pb.tile([P, 8, QW], BF, tag="attnT", bufs=3)
                for u in range(4):
                    sc_ps = ps.tile([P, 2, QW], F32, tag="sc", bufs=2)
                    for j in range(2):
                        vb = 2 * u + j
                        nc.tensor.matmul(
                            sc_ps[:, j, :],
                            kT_sb[:, h, b * LB + vb * P: b * LB + (vb + 1) * P],
                            qT_sb[:, h, qoff: qoff + QW],
                            start=True, stop=True)
                    nc.scalar.activation(
                        attnT[:, 2 * u:2 * u + 2, :], sc_ps[:],
                        mybir.ActivationFunctionType.Exp)
                S = pb.tile([P, QW], BF, tag="S", bufs=2)
                nc.vector.tensor_tensor(
                    out=S[:], in0=attnT[:, 0, :], in1=attnT[:, 1, :],
                    op=mybir.AluOpType.add)
                for vb in range(2, 8):
                    nc.vector.tensor_tensor(
                        out=S[:], in0=S[:], in1=attnT[:, vb, :],
                        op=mybir.AluOpType.add)
                return [b, h, lh, attnT, S, None]

            def b_stage2(st):
                b, S = st[0], st[4]
                rs_ps = ps.tile([1, QW], F32, tag="rs", bufs=1)
                nc.tensor.matmul(rs_ps[:], ones_bf[:], S[:],
                                 start=True, stop=True)
                rs_adj = pb.tile([1, QW], F32, tag="rsadj", bufs=2)
                nc.vector.tensor_scalar(
                    out=rs_adj[:], in0=rs_ps[:],
                    scalar1=nmask_sb[0:1, b:b + 1], scalar2=None,
                    op0=mybir.AluOpType.subtract)
                rcp_sb = pb.tile([1, QW], F32, tag="rcp", bufs=2)
                nc.vector.reciprocal_approx_fast(out=rcp_sb[:], in_=rs_adj[:])
                st[5] = rcp_sb

            def b_stage3(st):
                b, h, lh, attnT, S, rcp_sb = st
                qoff = b * LB + lh * QW
                rcp_ps = ps.tile([P, QW], F32, tag="rcpp", bufs=1)
                nc.tensor.matmul(rcp_ps[:], ones_f32[:], rcp_sb[:],
                                 start=True, stop=True)
                rcp_rep = pb.tile([P, QW], F32, tag="rcprep", bufs=1)
                nc.scalar.copy(out=rcp_rep[:], in_=rcp_ps[:])
                ctx_ps = ps.tile([P, QW], F32, tag="ctx", bufs=2)
                for vb in range(8):
                    nc.tensor.matmul(
                        ctx_ps[:],
                        v_sb[:, b * 8 + vb, h * P:(h + 1) * P],
                        attnT[:, vb, :],
                        start=(vb == 0), stop=(vb == 7))
                nc.vector.scalar_tensor_tensor(
                    out=ctxT_sb[:, h, qoff: qoff + QW],
                    in0=ctx_ps[:], scalar=CS, in1=rcp_rep[:],
                    op0=mybir.AluOpType.mult, op1=mybir.AluOpType.mult)

            pipe = []

            def b_push(st):
                pipe.append(st)
                if len(pipe) >= 2 and pipe[-2][5] is None:
                    b_stage2(pipe[-2])
                if len(pipe) >= 3:
                    b_stage3(pipe.pop(0))

            def b_drain():
                while pipe:
                    st = pipe.pop(0)
                    if st[5] is None:
                        b_stage2(st)
                    b_stage3(st)

            def a2a_send(b, lh):
                ch = b * 2 + lh
                for h in range(NHL):
                    nc.sync.dma_start(
                        out=a2a_in[ch][:, :, h, :].rearrange("j p l -> p j l"),
                        in_=ctxT_sb[:, h, b * LB + lh * QW: b * LB + (lh + 1) * QW]
                            .rearrange("p (j l) -> p j l", j=NCORES))
                nc.gpsimd.collective_compute(
                    "AllToAll", mybir.AluOpType.bypass,
                    replica_groups=[list(range(NCORES))],
                    ins=[a2a_in[ch][:]], outs=[a2a_out[ch][:]])

            def a2a_recv(b, lh):
                ch = b * 2 + lh
                if octxT[b] is None:
                    octxT[b] = pc.tile([P, KT, 2 * TLH], F8, name=f"octxT{b}")
                for i in range(NCORES):
                    nc.sync.dma_start(
                        out=octxT[b][:, i * NHL:(i + 1) * NHL,
                                     lh * TLH:(lh + 1) * TLH],
                        in_=a2a_out[ch][i])

            def phaseB(b):
                for lh in range(2):
                    for h in range(NHL):
                        b_push(b_stage1(b, h, lh))
                    b_drain()
                    a2a_send(b, lh)

            def phaseC(b, g_sb, bta_sb):
                # O-projection for this batch's 128 tokens, all H columns
                hb_h = []
                for u in range(2):
                    hb_sb = pc.tile([P, H // 2], F32, tag=f"hb{u}", bufs=1)
                    nc.sync.dma_start(
                        out=hb_sb[:],
                        in_=hb_d[b * P:(b + 1) * P,
                                 u * (H // 2):(u + 1) * (H // 2)])
                    hb_h.append(hb_sb)
                x = pc.tile([P, H], F32, tag="x", bufs=2)
                msum = pc.tile([P, 8], F32, tag="msum", bufs=2)
                qsum = pc.tile([P, 8], F32, tag="qsum", bufs=2)
                for mc in range(H // QW):
                    wo_h = []
                    for u in range(2):
                        wo_sb = pc.tile([P, KT // 2, QW], F8, tag="wo", bufs=3)
                        nc.sync.dma_start(
                            out=wo_sb[:],
                            in_=wo_d[:, mc, u * (KT // 2):(u + 1) * (KT // 2), :])
                        wo_h.append(wo_sb)
                    po = psC[0].tile([P, QW], F32, tag="po", bufs=3)
                    for t in range(NT):
                        u, tu = divmod(t, NT // 2)
                        nc.tensor.matmul(
                            po[:],
                            octxT[b][:, 2 * t:2 * t + 2, :],
                            wo_h[u][:, 2 * tu:2 * tu + 2, :],
                            start=(t == 0), stop=(t == NT - 1),
                            perf_mode=DR)
                    hbu = hb_h[mc // 4]
                    hcs = slice((mc % 4) * QW, (mc % 4 + 1) * QW)
                    nc.vector.scalar_tensor_tensor(
                        out=x[:, mc * QW:(mc + 1) * QW],
                        in0=po[:], scalar=1.0 / (WS * CS),
                        in1=hbu[:, hcs],
                        op0=mybir.AluOpType.mult, op1=mybir.AluOpType.add,
                        accum_out=msum[:, mc:mc + 1])
                    nc.scalar.activation(
                        hbu[:, hcs], x[:, mc * QW:(mc + 1) * QW],
                        mybir.ActivationFunctionType.Square,
                        accum_out=qsum[:, mc:mc + 1])
                if debug:
                    nc.sync.dma_start(
                        out=xpre_dbg[b * P:(b + 1) * P, :], in_=x[:])
                # ---- LayerNorm over H for the 128 tokens ----
                musum = pc.tile([P, 1], F32, tag="musum", bufs=2)
                nc.scalar.activation(
                    msum[:], msum[:], mybir.ActivationFunctionType.Copy,
                    accum_out=musum[:])
                mu = pc.tile([P, 1], F32, tag="mu", bufs=2)
                nc.scalar.mul(mu[:], musum[:], 1.0 / H)
                ssq = pc.tile([P, 1], F32, tag="ssq", bufs=2)
                nc.scalar.activation(
                    qsum[:], qsum[:], mybir.ActivationFunctionType.Copy,
                    accum_out=ssq[:])
                mu2 = pc.tile([P, 1], F32, tag="mu2", bufs=2)
                nc.scalar.activation(mu2[:], mu[:],
                                     mybir.ActivationFunctionType.Square)
                var = pc.tile([P, 1], F32, tag="var", bufs=2)
                nc.vector.scalar_tensor_tensor(
                    out=var[:], in0=ssq[:], scalar=1.0 / H, in1=mu2[:],
                    op0=mybir.AluOpType.mult, op1=mybir.AluOpType.subtract)
                eps_sb = pc.tile([P, 1], F32, tag="eps", bufs=1)
                nc.vector.memset(eps_sb[:], 1e-5)
                std = pc.tile([P, 1], F32, tag="std", bufs=2)
                nc.scalar.activation(std[:], var[:],
                                     mybir.ActivationFunctionType.Sqrt,
                                     bias=eps_sb[:], scale=1.0)
                rstd = pc.tile([P, 1], F32, tag="rstd", bufs=2)
                nc.vector.reciprocal(rstd[:], std[:])
                # normalize + gain + bias in column halves, overlapping the
                # output DMA with the second half's DVE passes
                HH = H // 2
                for c in range(2):
                    cs = slice(c * HH, (c + 1) * HH)
                    nc.vector.tensor_scalar(
                        out=x[:, cs], in0=x[:, cs], scalar1=mu[:],
                        scalar2=rstd[:],
                        op0=mybir.AluOpType.subtract, op1=mybir.AluOpType.mult)
                    nc.vector.tensor_tensor(
                        out=x[:, cs], in0=x[:, cs], in1=g_sb[:, cs],
                        op=mybir.AluOpType.mult)
                    nc.vector.tensor_tensor(
                        out=x[:, cs], in0=x[:, cs], in1=bta_sb[:, cs],
                        op=mybir.AluOpType.add)
                    nc.sync.dma_start(
                        out=out_d[b * P:(b + 1) * P, c * HH:(c + 1) * HH],
                        in_=x[:, cs])

            # g/bta: load one row each (staged through the hb-tag buffer),
            # broadcast on the (idle) GpSimd engine
            g_sb = pc.tile([P, H], F32, name="g_sb")
            bta_sb = pc.tile([P, H], F32, name="bta_sb")
            for u in range(2):
                stage = pc.tile([P, H // 2], F32, tag=f"hb{u}", bufs=1)
                src = g_d if u == 0 else bta_d
                dst = g_sb if u == 0 else bta_sb
                nc.sync.dma_start(out=stage[0:1, :H // 2], in_=src[0:1, :H // 2])
                nc.gpsimd.partition_broadcast(dst[:, :H // 2],
                                              stage[0:1, :H // 2])
                nc.sync.dma_start(out=stage[0:1, :H // 2],
                                  in_=src[0:1, H // 2:])
                nc.gpsimd.partition_broadcast(dst[:, H // 2:],
                                              stage[0:1, :H // 2])

            phaseB(0)
            a2a_recv(0, 0)
            a2a_recv(0, 1)
            phaseB(1)
            ps.release()
            psC[0] = tc.alloc_tile_pool(name="psC", bufs=1, space="PSUM")
            if debug:
                nc.sync.dma_start(
                    out=qT_dbg[:], in_=qT_sb[:].rearrange("p h l -> p (h l)"))
                nc.sync.dma_start(
                    out=kT_dbg[:], in_=kT_sb[:].rearrange("p h l -> p (h l)"))
                nc.sync.dma_start(
                    out=v_dbg[:], in_=v_sb[:].rearrange("p t c -> p (t c)"))
                for h in range(NHL):
                    ctmp = pers.tile([P, L2], F32, tag="ctmp", bufs=2)
                    nc.scalar.copy(out=ctmp[:], in_=ctxT_sb[:, h, :])
                    nc.sync.dma_start(
                        out=ctxT_dbg[:, h * L2:(h + 1) * L2], in_=ctmp[:])
            phaseC(0, g_sb, bta_sb)
            a2a_recv(1, 0)
            a2a_recv(1, 1)
            if debug:
                for b in range(B):
                    for kq in range(4):
                        otmp = pers.tile([P, 8 * 2 * TLH], F32, tag="otmp",
                                         bufs=2)
                        nc.scalar.copy(
                            out=otmp[:],
                            in_=octxT[b][:, kq * 8:(kq + 1) * 8, :]
                                .rearrange("p k l -> p (k l)"))
                        nc.sync.dma_start(
                            out=octxT_dbg[:, (b * KT + kq * 8) * 2 * TLH:
                                          (b * KT + (kq + 1) * 8) * 2 * TLH],
                            in_=otmp[:])
            phaseC(1, g_sb, bta_sb)
            psC[0].release()
            pc.release()
            pb.release()
            pqkv.release()

    nc.compile()
    return nc


def _prep_inputs(hidden_states, vision_features, attention_mask,
                 Wq, bq, Wk, bk, Wv, bv, Wo, bo, ln_g, ln_b):
    f = np.asarray
    hs = f(hidden_states, dtype=np.float32).reshape(L2, H)
    vf = f(vision_features, dtype=np.float32).reshape(L2, H)
    am = f(attention_mask)
    Wq, bq = f(Wq, dtype=np.float32), f(bq, dtype=np.float32)
    Wk, bk = f(Wk, dtype=np.float32), f(bk, dtype=np.float32)
    Wv, bv = f(Wv, dtype=np.float32), f(bv, dtype=np.float32)
    Wo, bo = f(Wo, dtype=np.float32), f(bo, dtype=np.float32)
    ln_g, ln_b = f(ln_g, dtype=np.float32), f(ln_b, dtype=np.float32)

    def act_layout(x):  # [L2, H] -> [P, NQ, KT, QW] fp8
        return np.ascontiguousarray(
            x.T.reshape(KT, P, NQ, QW).transpose(1, 2, 0, 3)).astype(FP8)

    def w_layout(w_slice):  # [CW, H] -> [P, KT, CW] fp8, pre-scaled
        return np.ascontiguousarray(
            (w_slice.T * WS).reshape(KT, P, CW).transpose(1, 0, 2)).astype(FP8)

    hid8 = act_layout(hs)
    vfm = vf.copy().reshape(B, LB, H)
    vfm[np.asarray(am) == 0] = 0.0          # masked vision tokens drop out
    vis8 = act_layout(vfm.reshape(L2, H))
    wo8 = np.ascontiguousarray(
        (Wo.T * WS).reshape(KT, P, H // QW, QW).transpose(1, 2, 0, 3)).astype(FP8)
    nmask = (np.asarray(am) == 0).sum(axis=1).astype(np.float32).reshape(1, B)
    bo_eff = bo + Wo @ bv

    in_maps = []
    for c in range(NCORES):
        sl = slice(c * CW, (c + 1) * CW)
        # core c owns tokens b*LB + lh*QW + c*TLH + [0,TLH) per (b, lh)
        hb = np.empty((B * 2 * TLH, H), np.float32)
        for b in range(B):
            for lh in range(2):
                rows = hs[b * LB + lh * QW + c * TLH:
                          b * LB + lh * QW + (c + 1) * TLH]
                hb[(b * 2 + lh) * TLH:(b * 2 + lh + 1) * TLH] = rows + bo_eff
        in_maps.append({
            "hid8": hid8,
            "vis8": vis8,
            "wq8": w_layout(Wq[sl]),
            "wk8": w_layout(Wk[sl]),
            "wv8": w_layout(Wv[sl]),
            "wo8": wo8,
            "bqT": np.ascontiguousarray((bq[sl] * SQ).reshape(NHL, P).T),
            "nmask": nmask,
            "hb": hb,
            "g": np.ascontiguousarray(ln_g.reshape(1, H)),
            "bta": np.ascontiguousarray(ln_b.reshape(1, H)),
        })
    return in_maps


def kernel(**inputs) -> np.ndarray:
    key = "dbg" if inputs.pop("_debug", False) else "main"
    if key not in _CACHE:
        _CACHE[key] = _build(debug=(key == "dbg"))
    nc = _CACHE[key]
    in_maps = _prep_inputs(**inputs)
    res = run_bass_kernel_spmd(nc, in_maps, list(range(NCORES)))
    out = np.empty((B, LB, H), np.float32)
    for c in range(NCORES):
        o = res.results[c]["out"]
        for b in range(B):
            for lh in range(2):
                out[b, lh * QW + c * TLH: lh * QW + (c + 1) * TLH] = \
                    o[(b * 2 + lh) * TLH:(b * 2 + lh + 1) * TLH]
    if key == "dbg":
        kernel._dbg = res.results
    return out
